# revision 70
# baseline (speedup 1.0000x reference)
"""Distributed Trainium2 kernel for the contrastive InfoNCE loss problem.

Strategy: shard the P = SY*SX = 275 position axis across 8 NeuronCores
(36 position slots per core, zero-padded + mask-corrected), logits in
[n, m] layout (n = anchor index on partitions, m = positive index on the
free axis) so the LSE reduction is a cheap free-axis DVE reduce:

  logits1 = G1^T @ lp'          G1  = Wl @ f^T          (static)
  logits2 = lt'^T @ (M @ lp0) + lt'^T @ BPP
            M   = Wl @ Wl^T     BPP = Wl @ biasP^T      (static)

where lt' = loc_t + solve(Wl^T, biasT^T) and lp' = loc_p +
solve(Wl^T, biasP^T) carry the join biases folded into the shipped fp16
data (so no on-chip bias adds), and lp0 is the plain loc_p for the
K = M @ lp0 path (keeps the M-cancellation well-conditioned).

Per super (2 positions): 11 matmuls (K, 2x logits1, 8x logits2), one
PSUM->SBUF fp16 conversion of K on DVE, one [128, 8x256] Exp activation
on ScalarE (PSUM in, bf16 SBUF out, bias = -SHIFT), one segmented DVE
reduce -> per-(position, n-chunk, loss) sums, and three DVE
tensor_tensor_reduce ops that produce all diagonal sums directly from
the SBUF operands (diag(A^T B) summed == sum(A*B)).  Epilogue: masked
ln-sum + diag totals -> one ones-matmul partition sum -> [4,1] DRAM.
Host sums the per-core partials.
"""

import numpy as np

# Problem constants (from the nn_ALL_9320079032780 spec).
N = 256
C = 128
SY, SX = 11, 25
P = SY * SX  # 275
D = 128
DM = 64
DC = 64
N_CORES = 8
POS_PER_CORE = 36  # padded; 18 supers of 2 positions
N_SUPERS = POS_PER_CORE // 2

SHIFT = 20.0

# packed fp16 statics layout: [G1 | M | BPP | Lt | Lp]
_H_G1 = 0
_H_M = _H_G1 + N
_H_BPP = _H_M + C
_H_LT = _H_BPP + N
_H_LP = _H_LT + N
STH_COLS = _H_LP + N
# packed f32 statics layout: [mask | shift | ones]
_F_MASK = 0
_F_SHIFT = _F_MASK + 8 * N_SUPERS
_F_ONES = _F_SHIFT + 1
STF_COLS = _F_ONES + 1

# per-super loc layout: [lt'(A) | lt'(B) | lp'(A) | lp'(B) | lp0(A) | lp0(B)]
LOC_COLS = 6 * N

_CACHED_NC = None


def _build_nc():
    import concourse.bass as bass  # noqa: F401
    import concourse.mybir as mybir
    import concourse.tile as tile
    from concourse import bacc

    f32 = mybir.dt.float32
    fp16 = mybir.dt.float16
    bf16 = mybir.dt.bfloat16
    Alu = mybir.AluOpType
    Act = mybir.ActivationFunctionType

    nc = bacc.Bacc("TRN2", target_bir_lowering=False, debug=False,
                   num_devices=N_CORES)

    # Make the act-table pass pick the combined exp+ln set so the kernel
    # pays a single ACT_TABLE_LOAD instead of one per function family.
    from concourse.hw_specs import get_activation_tables
    _tabs = get_activation_tables(nc.m.arch)
    _Exp, _Ln = Act.Exp, Act.Ln
    for _name, _fns in _tabs.items():
        if _name != "natural_log_exp_and_others":
            _fns.discard(_Exp)
            _fns.discard(_Ln)

    loc_d = nc.declare_dram_parameter("loc", [N_SUPERS, 128, LOC_COLS], fp16,
                                      isOutput=False)
    sth_d = nc.declare_dram_parameter("sth", [128, STH_COLS], fp16,
                                      isOutput=False)
    stf_d = nc.declare_dram_parameter("stf", [128, STF_COLS], f32,
                                      isOutput=False)
    out_d = nc.declare_dram_parameter("out", [1, 6], f32, isOutput=True)

    with tile.TileContext(nc) as tc:
        with (
            tc.tile_pool(name="statics", bufs=1) as st,
            tc.tile_pool(name="loc", bufs=6) as locpool,
            tc.tile_pool(name="ksb", bufs=5) as kpool,
            tc.tile_pool(name="et", bufs=6) as etpool,
            tc.tile_pool(name="etf", bufs=4) as etfpool,
            tc.tile_pool(name="scr", bufs=2) as scrpool,
            tc.tile_pool(name="lga", bufs=2, space="PSUM") as lgap,
            tc.tile_pool(name="lgb", bufs=2, space="PSUM") as lgbp,
        ):
            # ---- statics: two packed DMAs on the scalar queue ----
            sth = st.tile([128, STH_COLS], fp16, tag="sth")
            stf = st.tile([128, STF_COLS], f32, tag="stf")
            nc.sync.dma_start(out=sth[:, :], in_=sth_d[:, :])
            nc.scalar.dma_start(out=stf[:, :], in_=stf_d[:, :])
            G1 = sth[:, _H_G1:_H_G1 + N]
            Mst = sth[:, _H_M:_H_M + C]
            BPP = sth[:, _H_BPP:_H_BPP + N]
            Lt = sth[:, _H_LT:_H_LT + N]
            Lp = sth[:, _H_LP:_H_LP + N]
            mask = stf[:, _F_MASK:_F_MASK + 8 * N_SUPERS]
            shiftc = stf[:, _F_SHIFT:_F_SHIFT + 1]
            onesc = stf[:, _F_ONES:_F_ONES + 1]

            # persistent accumulators (bf16 lse sums: 2-byte dst enables the
            # DVE 2x perf mode on the reduce; rounding is within tolerance)
            Sacc = st.tile([128, 8 * N_SUPERS], bf16, tag="Sacc")
            dcols = st.tile([128, N_SUPERS], f32, tag="dcols")
            lnS = st.tile([128, 8 * N_SUPERS], f32, tag="lnS")
            scrln = st.tile([128, 8 * N_SUPERS], f32, tag="scrln")
            scr2 = st.tile([128, N], fp16, tag="scr2")
            pack = st.tile([128, 6], f32, tag="pack")

            # HAM warmup: dummy matmuls keep the PE busy/ramping through
            # the cold window while the first DMAs land.
            wtile = st.tile([128, 128], fp16, tag="wtile")
            nc.vector.memset(wtile[:, :], 0.0)
            wps = lgap.tile([128, 1024], f32, tag="lga")
            for _w in range(3):
                nc.tensor.matmul(
                    out=wps[:, 0:512].rearrange("p (k n) -> p k n", k=4),
                    lhsT=wtile,
                    rhs=wtile[:, :].unsqueeze(1).broadcast_to([128, 4, 128]),
                    start=True, stop=True)
            # prime the act table (exp+ln combined set) with no dependency
            # on the statics DMA so the ~1.3us load runs during the fill.
            prim = st.tile([128, 1], f32, tag="prim")
            nc.scalar.activation(prim[:, :], wtile[:, 0:1], Act.Exp,
                                 bias=0.0)

            def stage_dma(s, split=False):
                lpt = locpool.tile([128, LOC_COLS], fp16, tag="lpt")
                if split:
                    # fill optimization: land the two halves on parallel
                    # queues so mmK(0)/l1(0) start earlier.
                    nc.sync.dma_start(out=lpt[:, 0:4 * N],
                                      in_=loc_d[s, :, 0:4 * N])
                    nc.scalar.dma_start(out=lpt[:, 4 * N:6 * N],
                                        in_=loc_d[s, :, 4 * N:6 * N])
                else:
                    nc.sync.dma_start(out=lpt[:, :], in_=loc_d[s, :, :])
                return lpt

            def emit_k_mm(s, lpt):
                # K = M @ lp0 into bank0 of this super's lgB tile — l2
                # overwrites it only after the cast read it, and l2 depends
                # on the cast output anyway, so no extra serialization.
                lgb = lgbp.tile([128, 1024], f32, tag="lgb")
                nc.tensor.matmul(out=lgb[:, 0:512], lhsT=Mst,
                                 rhs=lpt[:, 4 * N:6 * N], start=True,
                                 stop=True)
                return lgb

            def emit_cast(s, lgb, on_act):
                # K PSUM -> fp16 SBUF (split DVE / ScalarE by super).
                ksb = kpool.tile([128, 512], fp16, tag="ksb")
                if on_act:
                    nc.scalar.copy(out=ksb[:, :], in_=lgb[:, 0:512])
                else:
                    nc.vector.tensor_copy(out=ksb[:, :], in_=lgb[:, 0:512])
                return ksb

            def emit_l1(s, lpt):
                # logits1: lhsT = G1 chunk, rhs = lp' (both positions).
                lga = lgap.tile([128, 1024], f32, tag="lga")
                lp = lpt[:, 2 * N:4 * N]
                for h in range(2):
                    nc.tensor.matmul(out=lga[:, h * 512:(h + 1) * 512],
                                     lhsT=G1[:, h * 128:(h + 1) * 128],
                                     rhs=lp, start=True, stop=True)
                return lga

            def emit_l2(s, lpt, lgb, ksb, k):
                # logits2 position k: lhsT = lt' n-chunk, rhs = K then BPP.
                for h in range(2):
                    ob = lgb[:, h * 512 + k * 256:h * 512 + (k + 1) * 256]
                    lh = lpt[:, k * N + h * 128:k * N + (h + 1) * 128]
                    nc.tensor.matmul(out=ob, lhsT=lh,
                                     rhs=ksb[:, k * N:(k + 1) * N],
                                     start=True, stop=False)
                    nc.tensor.matmul(out=ob, lhsT=lh, rhs=BPP,
                                     start=False, stop=True)

            def emit_exp_a(s, lga):
                et = etpool.tile([128, 2048], bf16, tag="et")
                nc.scalar.activation(et[:, 0:1024], lga[:, :], Act.Exp,
                                     bias=shiftc[:, 0:1])
                return et

            def emit_exp_b(s, lgb, et):
                nc.scalar.activation(et[:, 1024:2048], lgb[:, :], Act.Exp,
                                     bias=shiftc[:, 0:1])

            def emit_fold(s, et):
                # GpSimd folds the two 128-halves of each block, halving the
                # DVE reduce width; every 3rd super gets a second fold.
                etf = etfpool.tile([128, 1024], bf16, tag="etf")
                ev = et[:, :].rearrange("p (q t m) -> p q t m", q=8, t=2)
                nc.gpsimd.tensor_tensor(
                    out=etf[:, :].rearrange("p (q m) -> p q m", q=8),
                    in0=ev[:, :, 0, :], in1=ev[:, :, 1, :], op=Alu.add)
                return (etf, 128)

            def emit_red(s, etf_m):
                etf, m = etf_m
                with nc.allow_low_precision("bf16 lse sums within tolerance"):
                    nc.vector.tensor_reduce(
                        out=Sacc[:, 8 * s:8 * (s + 1)],
                        in_=etf[:, :].rearrange("p (q m) -> p q m", q=8),
                        axis=mybir.AxisListType.X, op=Alu.add)

            def emit_diags(s, lpt, ksb):
                # diag2 K-part via sum(lt' * K) == sum(diag(pred^T pos_nb));
                # the static-side diag sums (lt'*BPP, lp'*G1) are hoisted to
                # the host (Lt/Lp statics, epilogue).
                scr = scrpool.tile([128, 512], fp16, tag="scr")
                nc.vector.scalar_tensor_tensor(
                    out=scr[:, :], in0=lpt[:, 0:2 * N], scalar=1.0,
                    in1=ksb[:, :], op0=Alu.mult, op1=Alu.mult,
                    accum_out=dcols[:, s:s + 1])

            # ---- main loop, software-pipelined ----
            # PE order per iter: mmK(s+1) first (its lgA buffer was freed by
            # expA(s-1) early last iter), then l1(s), l2(s) — so the cast of
            # K(s+1) has a full iteration of slack before l1(s+1) needs it.
            lpt_cur = stage_dma(0, split=True)
            lpt_nxt = stage_dma(1)
            lgb_cur = emit_k_mm(0, lpt_cur)
            ksb_cur = emit_cast(0, lgb_cur, on_act=False)
            etfs = []
            for s in range(N_SUPERS):
                if s + 2 < N_SUPERS:
                    lpt_fut = stage_dma(s + 2)
                lga_cur = emit_l1(s, lpt_cur)
                et_cur = emit_exp_a(s, lga_cur)
                if s + 1 < N_SUPERS:
                    lgb_nxt = emit_k_mm(s + 1, lpt_nxt)
                    ksb_nxt = emit_cast(s + 1, lgb_nxt,
                                        on_act=(s % 2 == 0))
                emit_l2(s, lpt_cur, lgb_cur, ksb_cur, 0)
                emit_l2(s, lpt_cur, lgb_cur, ksb_cur, 1)
                emit_exp_b(s, lgb_cur, et_cur)
                # reduce before the diag stt: the 1-port reduce is immune to
                # the GpSimd SBUF-port contention while the fold runs; the
                # 2-port stt is lagged one super so it lands after the
                # fold's contention window.
                if len(etfs) >= 2:
                    emit_red(s - 2, etfs[-2])
                if s >= 1:
                    emit_diags(s - 1, lpt_prv, ksb_prv)
                lpt_prv, ksb_prv = lpt_cur, ksb_cur
                if s < N_SUPERS - 1:
                    etfs.append(emit_fold(s, et_cur))
                else:
                    et_last = et_cur
                if s == N_SUPERS - 2:
                    # partial epilogue for supers 0..13 (finalized by
                    # red(13), emitted at s=15) overlaps the last supers.
                    SA = 8 * 14
                    nc.scalar.activation(lnS[:, 0:SA], Sacc[:, 0:SA],
                                         Act.Ln)
                    nc.vector.scalar_tensor_tensor(
                        out=scrln[:, 0:SA], in0=lnS[:, 0:SA], scalar=1.0,
                        in1=mask[:, 0:SA], op0=Alu.mult, op1=Alu.mult,
                        accum_out=pack[:, 0:1])
                    nc.vector.scalar_tensor_tensor(
                        out=scr2[:, :], in0=Lt, scalar=1.0, in1=BPP,
                        op0=Alu.mult, op1=Alu.mult, accum_out=pack[:, 4:5])
                    nc.vector.scalar_tensor_tensor(
                        out=scr2[:, :], in0=Lp, scalar=1.0, in1=G1,
                        op0=Alu.mult, op1=Alu.mult, accum_out=pack[:, 5:6])
                if s + 1 < N_SUPERS:
                    lpt_cur, lgb_cur, ksb_cur = lpt_nxt, lgb_nxt, ksb_nxt
                    if s + 2 < N_SUPERS:
                        lpt_nxt = lpt_fut
            emit_diags(N_SUPERS - 1, lpt_prv, ksb_prv)
            emit_red(N_SUPERS - 2, etfs[-1])
            # last super: direct (unfolded) reduce in two halves to skip the
            # Pool hop in the drain; the A half overlaps expB(17).
            S17 = 8 * (N_SUPERS - 1)
            with nc.allow_low_precision("bf16 lse sums within tolerance"):
                nc.vector.tensor_reduce(
                    out=Sacc[:, S17:S17 + 4],
                    in_=et_last[:, 0:1024].rearrange("p (q m) -> p q m", q=4),
                    axis=mybir.AxisListType.X, op=Alu.add)
                # lnB1 covers supers 14-16 plus super 17's l1 blocks — only
                # the last 4 l2 columns serialize behind the final reduce.
                SA = 8 * 14
                nc.scalar.activation(lnS[:, SA:S17 + 4], Sacc[:, SA:S17 + 4],
                                     Act.Ln)
                nc.vector.tensor_reduce(
                    out=Sacc[:, S17 + 4:S17 + 8],
                    in_=et_last[:, 1024:2048].rearrange("p (q m) -> p q m",
                                                        q=4),
                    axis=mybir.AxisListType.X, op=Alu.add)
            nc.vector.scalar_tensor_tensor(
                out=scrln[:, SA:S17 + 4], in0=lnS[:, SA:S17 + 4], scalar=1.0,
                in1=mask[:, SA:S17 + 4], op0=Alu.mult, op1=Alu.mult,
                accum_out=pack[:, 1:2])
            nc.scalar.activation(lnS[:, S17 + 4:S17 + 8],
                                 Sacc[:, S17 + 4:S17 + 8], Act.Ln)
            nc.vector.tensor_reduce(
                out=pack[:, 3:4], in_=dcols[:, :],
                axis=mybir.AxisListType.X, op=Alu.add)
            psF = lgap.tile([128, 1024], f32, tag="lga")
            for j in (0, 1, 3, 4, 5):
                nc.tensor.matmul(out=psF[0:1, j:j + 1],
                                 lhsT=pack[:, j:j + 1],
                                 rhs=onesc[:, 0:1], start=True, stop=True)
            nc.vector.scalar_tensor_tensor(
                out=scrln[:, S17 + 4:S17 + 8], in0=lnS[:, S17 + 4:S17 + 8],
                scalar=1.0, in1=mask[:, S17 + 4:S17 + 8], op0=Alu.mult,
                op1=Alu.mult, accum_out=pack[:, 2:3])
            nc.tensor.matmul(out=psF[0:1, 2:3], lhsT=pack[:, 2:3],
                             rhs=onesc[:, 0:1], start=True, stop=True)
            out_sb = st.tile([1, 8], f32, tag="out_sb")
            nc.vector.tensor_copy(out=out_sb[0:1, 0:6], in_=psF[0:1, 0:6])
            nc.sync.dma_start(out=out_d[:, :], in_=out_sb[0:1, 0:6])

    nc.finalize()
    return nc


def _get_nc():
    global _CACHED_NC
    if _CACHED_NC is None:
        _CACHED_NC = _build_nc()
    return _CACHED_NC


def _core_position_lists():
    """275 positions -> 8 cores: 3 cores x 35, 5 cores x 34."""
    lists = []
    start = 0
    for i in range(N_CORES):
        cnt = 35 if i < 3 else 34
        lists.append(list(range(start, start + cnt)))
        start += cnt
    assert start == P
    return lists


def _prep_in_maps(f_t_global, x_t_local, x_t_prev_local, m_t, m_t_prev, c_t,
                  c_t_prev, W_join, b_join):
    W = W_join.astype(np.float64)
    Wl, Wm, Wc = W[:C], W[C:C + DM], W[C + DM:]
    biasP = (m_t_prev.astype(np.float64) @ Wm
             + c_t_prev.astype(np.float64) @ Wc + b_join)
    biasT = (m_t.astype(np.float64) @ Wm
             + c_t.astype(np.float64) @ Wc + b_join)
    dP = np.linalg.solve(Wl.T, biasP.T)  # [C, N]
    dT = np.linalg.solve(Wl.T, biasT.T)

    sth0 = np.zeros((128, STH_COLS), dtype=np.float16)
    sth0[:, _H_G1:_H_G1 + N] = (Wl @ f_t_global.astype(np.float64).T
                                ).astype(np.float16)
    sth0[:, _H_M:_H_M + C] = (Wl @ Wl.T).astype(np.float16)
    sth0[:, _H_BPP:_H_BPP + N] = (Wl @ biasP.T).astype(np.float16)

    # [N, C, SY, SX] -> [P, C, N]
    locp = np.ascontiguousarray(
        x_t_prev_local.reshape(N, C, P).transpose(2, 1, 0))
    loct = np.ascontiguousarray(
        x_t_local.reshape(N, C, P).transpose(2, 1, 0))
    lt_ = (loct + dT[None].astype(np.float32)).astype(np.float16)
    lp_ = (locp + dP[None].astype(np.float32)).astype(np.float16)
    lp0 = locp.astype(np.float16)

    in_maps = []
    for ids in _core_position_lists():
        npos = len(ids)
        loc = np.zeros((N_SUPERS, 128, LOC_COLS), dtype=np.float16)
        for j, p in enumerate(ids):
            s, k = divmod(j, 2)
            loc[s, :, k * N:(k + 1) * N] = lt_[p]
            loc[s, :, 2 * N + k * N:2 * N + (k + 1) * N] = lp_[p]
            loc[s, :, 4 * N + k * N:4 * N + (k + 1) * N] = lp0[p]
        stf = np.zeros((128, STF_COLS), dtype=np.float32)
        # mask col (s*8 + b): block b is position 2s + (b & 1)
        for s in range(N_SUPERS):
            for b in range(8):
                if 2 * s + (b & 1) < npos:
                    stf[:, _F_MASK + 8 * s + b] = 1.0
        stf[:, _F_SHIFT] = -SHIFT
        stf[:, _F_ONES] = 1.0
        # hoisted diag-sum operands: position sums of the shipped fp16 data
        sth = sth0.copy()
        sth[:, _H_LT:_H_LT + N] = (lt_[ids].astype(np.float32).sum(axis=0)
                                   ).astype(np.float16)
        sth[:, _H_LP:_H_LP + N] = (lp_[ids].astype(np.float32).sum(axis=0)
                                   ).astype(np.float16)
        in_maps.append({"loc": loc, "sth": sth, "stf": stf})
    return in_maps


def kernel(f_t_global, x_t_local, x_t_prev_local, m_t, m_t_prev, c_t,
           c_t_prev, W_join, b_join):
    from concourse.bass_utils import run_bass_kernel_spmd

    args = [f_t_global, x_t_local, x_t_prev_local, m_t, m_t_prev, c_t,
            c_t_prev, W_join, b_join]
    args = [np.asarray(a, dtype=np.float32) for a in args]
    in_maps = _prep_in_maps(*args)
    nc = _get_nc()
    res = run_bass_kernel_spmd(nc, in_maps, core_ids=list(range(N_CORES)))
    return combine(res)


def combine(res):
    """Host-side reduction of the 8 per-core [4, 1] partials."""
    total = 0.0
    for i, ids in enumerate(_core_position_lists()):
        v = res.results[i]["out"].reshape(-1)
        npos = len(ids)
        # v[0:3] = masked ln(S) pieces = sum(lse - SHIFT); v[3:6] = diag
        # sums (d2a, d2b, d1)
        total += (float(v[0]) + float(v[1]) + float(v[2])
                  + SHIFT * 2 * N * npos
                  - float(v[3]) - float(v[4]) - float(v[5]))
    return np.asarray(total / (P * N), dtype=np.float32)


# revision 79
# speedup vs baseline: 1.0481x; 1.0481x over previous
"""Distributed Trainium2 kernel for the contrastive InfoNCE loss problem.

Strategy: shard the P = SY*SX = 275 position axis across 8 NeuronCores
(36 position slots per core, zero-padded + mask-corrected), logits in
[n, m] layout (n = anchor index on partitions, m = positive index on the
free axis) so the LSE reduction is a cheap free-axis DVE reduce:

  logits1 = G1^T @ lp'          G1  = Wl @ f^T          (static)
  logits2 = lt'^T @ (M @ lp0) + lt'^T @ BPP
            M   = Wl @ Wl^T     BPP = Wl @ biasP^T      (static)

where lt' = loc_t + solve(Wl^T, biasT^T) and lp' = loc_p +
solve(Wl^T, biasP^T) carry the join biases folded into the shipped fp16
data (so no on-chip bias adds), and lp0 is the plain loc_p for the
K = M @ lp0 path (keeps the M-cancellation well-conditioned).

Per super (2 positions): 11 matmuls (K, 2x logits1, 8x logits2), one
PSUM->SBUF fp16 conversion of K on DVE, one [128, 8x256] Exp activation
on ScalarE (PSUM in, bf16 SBUF out, bias = -SHIFT), one segmented DVE
reduce -> per-(position, n-chunk, loss) sums, and three DVE
tensor_tensor_reduce ops that produce all diagonal sums directly from
the SBUF operands (diag(A^T B) summed == sum(A*B)).  Epilogue: masked
ln-sum + diag totals -> one ones-matmul partition sum -> [4,1] DRAM.
Host sums the per-core partials.
"""

import numpy as np

# Problem constants (from the nn_ALL_9320079032780 spec).
N = 256
C = 128
SY, SX = 11, 25
P = SY * SX  # 275
D = 128
DM = 64
DC = 64
N_CORES = 8
POS_PER_CORE = 36  # padded; 18 supers of 2 positions
N_SUPERS = POS_PER_CORE // 2

SHIFT = 20.0

# packed fp16 statics layout: [G1 | M | BPP | Lt | Lp]
_H_G1 = 0
_H_M = _H_G1 + N
_H_BPP = _H_M + C
_H_LT = _H_BPP + N
_H_LP = _H_LT + N
STH_COLS = _H_LP + N
# packed f32 statics layout: [mask | shift | ones]
_F_MASK = 0
_F_SHIFT = _F_MASK + 8 * N_SUPERS
_F_ONES = _F_SHIFT + 1
STF_COLS = _F_ONES + 1

# per-super loc layout: [lt'(A) | lt'(B) | lp'(A) | lp'(B) | lp0(A) | lp0(B)]
LOC_COLS = 6 * N

_CACHED_NC = None


def _build_nc():
    import concourse.bass as bass  # noqa: F401
    import concourse.mybir as mybir
    import concourse.tile as tile
    from concourse import bacc

    f32 = mybir.dt.float32
    fp16 = mybir.dt.float16
    bf16 = mybir.dt.bfloat16
    Alu = mybir.AluOpType
    Act = mybir.ActivationFunctionType

    nc = bacc.Bacc("TRN2", target_bir_lowering=False, debug=False,
                   num_devices=N_CORES)

    # Make the act-table pass pick the combined exp+ln set so the kernel
    # pays a single ACT_TABLE_LOAD instead of one per function family.
    from concourse.hw_specs import get_activation_tables
    _tabs = get_activation_tables(nc.m.arch)
    _Exp, _Ln = Act.Exp, Act.Ln
    for _name, _fns in _tabs.items():
        if _name != "natural_log_exp_and_others":
            _fns.discard(_Exp)
            _fns.discard(_Ln)

    loc_d = nc.declare_dram_parameter("loc", [N_SUPERS, 128, LOC_COLS], fp16,
                                      isOutput=False)
    # sth and super-0's loc tile ride one DMA: a single DGE latency +
    # completion semaphore on the critical startup chain instead of two.
    sth_d = nc.declare_dram_parameter("sth", [128, STH_COLS + LOC_COLS],
                                      fp16, isOutput=False)
    stf_d = nc.declare_dram_parameter("stf", [128, STF_COLS], f32,
                                      isOutput=False)
    out_d = nc.declare_dram_parameter("out", [1, 6], f32, isOutput=True)

    with tile.TileContext(nc) as tc:
        with (
            tc.tile_pool(name="statics", bufs=1) as st,
            tc.tile_pool(name="loc", bufs=6) as locpool,
            tc.tile_pool(name="ksb", bufs=5) as kpool,
            tc.tile_pool(name="et", bufs=6) as etpool,
            tc.tile_pool(name="etf", bufs=4) as etfpool,
            tc.tile_pool(name="scr", bufs=2) as scrpool,
            tc.tile_pool(name="lga", bufs=2, space="PSUM") as lgap,
            tc.tile_pool(name="lgb", bufs=2, space="PSUM") as lgbp,
        ):
            # ---- statics: two packed DMAs on the scalar queue ----
            sth = st.tile([128, STH_COLS + LOC_COLS], fp16, tag="sth")
            stf = st.tile([128, STF_COLS], f32, tag="stf")
            nc.sync.dma_start(out=sth[:, :], in_=sth_d[:, :])
            nc.scalar.dma_start(out=stf[:, :], in_=stf_d[:, :])
            loc0 = sth[:, STH_COLS:STH_COLS + LOC_COLS]
            G1 = sth[:, _H_G1:_H_G1 + N]
            Mst = sth[:, _H_M:_H_M + C]
            BPP = sth[:, _H_BPP:_H_BPP + N]
            Lt = sth[:, _H_LT:_H_LT + N]
            Lp = sth[:, _H_LP:_H_LP + N]
            mask = stf[:, _F_MASK:_F_MASK + 8 * N_SUPERS]
            shiftc = stf[:, _F_SHIFT:_F_SHIFT + 1]
            onesc = stf[:, _F_ONES:_F_ONES + 1]

            # persistent accumulators (bf16 lse sums: 2-byte dst enables the
            # DVE 2x perf mode on the reduce; rounding is within tolerance)
            Sacc = st.tile([128, 8 * N_SUPERS], bf16, tag="Sacc")
            dcols = st.tile([128, N_SUPERS], f32, tag="dcols")
            lnS = st.tile([128, 8 * N_SUPERS], f32, tag="lnS")
            scrln = st.tile([128, 8 * N_SUPERS], f32, tag="scrln")
            scr2 = st.tile([128, N], fp16, tag="scr2")
            pack = st.tile([128, 6], f32, tag="pack")

            # HAM warmup: dummy matmuls keep the PE busy/ramping through
            # the cold window while the first DMAs land.
            wtile = st.tile([128, 128], fp16, tag="wtile")
            nc.vector.memset(wtile[:, :], 0.0)
            wps = lgap.tile([128, 1024], f32, tag="lga")
            for _w in range(3):
                nc.tensor.matmul(
                    out=wps[:, 0:512].rearrange("p (k n) -> p k n", k=4),
                    lhsT=wtile,
                    rhs=wtile[:, :].unsqueeze(1).broadcast_to([128, 4, 128]),
                    start=True, stop=True)
            # prime the act table (exp+ln combined set) with no dependency
            # on the statics DMA so the ~1.3us load runs during the fill.
            prim = st.tile([128, 1], f32, tag="prim")
            nc.scalar.activation(prim[:, :], wtile[:, 0:1], Act.Exp,
                                 bias=0.0)

            def stage_dma(s):
                lpt = locpool.tile([128, LOC_COLS], fp16, tag="lpt")
                nc.sync.dma_start(out=lpt[:, :], in_=loc_d[s, :, :])
                return lpt

            def emit_k_mm(s, lpt):
                # K = M @ lp0 into bank0 of this super's lgB tile — l2
                # overwrites it only after the cast read it, and l2 depends
                # on the cast output anyway, so no extra serialization.
                lgb = lgbp.tile([128, 1024], f32, tag="lgb")
                nc.tensor.matmul(out=lgb[:, 0:512], lhsT=Mst,
                                 rhs=lpt[:, 4 * N:6 * N], start=True,
                                 stop=True)
                return lgb

            def emit_cast(s, lgb, on_act):
                # K PSUM -> fp16 SBUF (split DVE / ScalarE by super).
                ksb = kpool.tile([128, 512], fp16, tag="ksb")
                if on_act:
                    nc.scalar.copy(out=ksb[:, :], in_=lgb[:, 0:512])
                else:
                    nc.vector.tensor_copy(out=ksb[:, :], in_=lgb[:, 0:512])
                return ksb

            def emit_l1(s, lpt):
                # logits1: lhsT = G1 chunk, rhs = lp' (both positions).
                lga = lgap.tile([128, 1024], f32, tag="lga")
                lp = lpt[:, 2 * N:4 * N]
                for h in range(2):
                    nc.tensor.matmul(out=lga[:, h * 512:(h + 1) * 512],
                                     lhsT=G1[:, h * 128:(h + 1) * 128],
                                     rhs=lp, start=True, stop=True)
                return lga

            def emit_l2(s, lpt, lgb, ksb, k):
                # logits2 position k: lhsT = lt' n-chunk, rhs = K then BPP.
                for h in range(2):
                    ob = lgb[:, h * 512 + k * 256:h * 512 + (k + 1) * 256]
                    lh = lpt[:, k * N + h * 128:k * N + (h + 1) * 128]
                    nc.tensor.matmul(out=ob, lhsT=lh,
                                     rhs=ksb[:, k * N:(k + 1) * N],
                                     start=True, stop=False)
                    nc.tensor.matmul(out=ob, lhsT=lh, rhs=BPP,
                                     start=False, stop=True)

            def emit_exp_a(s, lga):
                et = etpool.tile([128, 2048], bf16, tag="et")
                nc.scalar.activation(et[:, 0:1024], lga[:, :], Act.Exp,
                                     bias=shiftc[:, 0:1])
                return et

            def emit_exp_b(s, lgb, et):
                nc.scalar.activation(et[:, 1024:2048], lgb[:, :], Act.Exp,
                                     bias=shiftc[:, 0:1])

            def emit_fold(s, et):
                # GpSimd folds the two 128-halves of each block, halving the
                # DVE reduce width; every 3rd super gets a second fold.
                etf = etfpool.tile([128, 1024], bf16, tag="etf")
                ev = et[:, :].rearrange("p (q t m) -> p q t m", q=8, t=2)
                nc.gpsimd.tensor_tensor(
                    out=etf[:, :].rearrange("p (q m) -> p q m", q=8),
                    in0=ev[:, :, 0, :], in1=ev[:, :, 1, :], op=Alu.add)
                return (etf, 128)

            def emit_red(s, etf_m):
                etf, m = etf_m
                with nc.allow_low_precision("bf16 lse sums within tolerance"):
                    nc.vector.tensor_reduce(
                        out=Sacc[:, 8 * s:8 * (s + 1)],
                        in_=etf[:, :].rearrange("p (q m) -> p q m", q=8),
                        axis=mybir.AxisListType.X, op=Alu.add)

            def emit_diags(s, lpt, ksb):
                # diag2 K-part via sum(lt' * K) == sum(diag(pred^T pos_nb));
                # the static-side diag sums (lt'*BPP, lp'*G1) are hoisted to
                # the host (Lt/Lp statics, epilogue).
                scr = scrpool.tile([128, 512], fp16, tag="scr")
                nc.vector.scalar_tensor_tensor(
                    out=scr[:, :], in0=lpt[:, 0:2 * N], scalar=1.0,
                    in1=ksb[:, :], op0=Alu.mult, op1=Alu.mult,
                    accum_out=dcols[:, s:s + 1])

            # ---- main loop, software-pipelined ----
            # PE order per iter: mmK(s+1) first (its lgA buffer was freed by
            # expA(s-1) early last iter), then l1(s), l2(s) — so the cast of
            # K(s+1) has a full iteration of slack before l1(s+1) needs it.
            lpt_cur = loc0
            lpt_nxt = stage_dma(1)
            lgb_cur = emit_k_mm(0, lpt_cur)
            ksb_cur = emit_cast(0, lgb_cur, on_act=False)
            etfs = []
            for s in range(N_SUPERS):
                if s + 2 < N_SUPERS:
                    lpt_fut = stage_dma(s + 2)
                lga_cur = emit_l1(s, lpt_cur)
                et_cur = emit_exp_a(s, lga_cur)
                if s + 1 < N_SUPERS:
                    lgb_nxt = emit_k_mm(s + 1, lpt_nxt)
                    ksb_nxt = emit_cast(s + 1, lgb_nxt,
                                        on_act=(s % 2 == 0))
                emit_l2(s, lpt_cur, lgb_cur, ksb_cur, 0)
                emit_l2(s, lpt_cur, lgb_cur, ksb_cur, 1)
                emit_exp_b(s, lgb_cur, et_cur)
                # reduce before the diag stt: the 1-port reduce is immune to
                # the GpSimd SBUF-port contention while the fold runs; the
                # 2-port stt is lagged one super so it lands after the
                # fold's contention window.
                if len(etfs) >= 2:
                    emit_red(s - 2, etfs[-2])
                if s >= 1:
                    emit_diags(s - 1, lpt_prv, ksb_prv)
                lpt_prv, ksb_prv = lpt_cur, ksb_cur
                if s < N_SUPERS - 1:
                    etfs.append(emit_fold(s, et_cur))
                else:
                    et_last = et_cur
                if s == N_SUPERS - 2:
                    # partial epilogue for supers 0..13 (finalized by
                    # red(13), emitted at s=15) overlaps the last supers.
                    SA = 8 * 14
                    nc.scalar.activation(lnS[:, 0:SA], Sacc[:, 0:SA],
                                         Act.Ln)
                    nc.vector.scalar_tensor_tensor(
                        out=scrln[:, 0:SA], in0=lnS[:, 0:SA], scalar=1.0,
                        in1=mask[:, 0:SA], op0=Alu.mult, op1=Alu.mult,
                        accum_out=pack[:, 0:1])
                    nc.vector.scalar_tensor_tensor(
                        out=scr2[:, :], in0=Lt, scalar=1.0, in1=BPP,
                        op0=Alu.mult, op1=Alu.mult, accum_out=pack[:, 4:5])
                    nc.vector.scalar_tensor_tensor(
                        out=scr2[:, :], in0=Lp, scalar=1.0, in1=G1,
                        op0=Alu.mult, op1=Alu.mult, accum_out=pack[:, 5:6])
                if s + 1 < N_SUPERS:
                    lpt_cur, lgb_cur, ksb_cur = lpt_nxt, lgb_nxt, ksb_nxt
                    if s + 2 < N_SUPERS:
                        lpt_nxt = lpt_fut
            emit_diags(N_SUPERS - 1, lpt_prv, ksb_prv)
            emit_red(N_SUPERS - 2, etfs[-1])
            # last super: direct (unfolded) reduce in two halves to skip the
            # Pool hop in the drain; the A half overlaps expB(17).
            S17 = 8 * (N_SUPERS - 1)
            with nc.allow_low_precision("bf16 lse sums within tolerance"):
                nc.vector.tensor_reduce(
                    out=Sacc[:, S17:S17 + 4],
                    in_=et_last[:, 0:1024].rearrange("p (q m) -> p q m", q=4),
                    axis=mybir.AxisListType.X, op=Alu.add)
                # lnB1 covers supers 14-16 plus super 17's l1 blocks — only
                # the last 4 l2 columns serialize behind the final reduce.
                SA = 8 * 14
                nc.scalar.activation(lnS[:, SA:S17 + 4], Sacc[:, SA:S17 + 4],
                                     Act.Ln)
                nc.vector.tensor_reduce(
                    out=Sacc[:, S17 + 4:S17 + 8],
                    in_=et_last[:, 1024:2048].rearrange("p (q m) -> p q m",
                                                        q=4),
                    axis=mybir.AxisListType.X, op=Alu.add)
            nc.vector.scalar_tensor_tensor(
                out=scrln[:, SA:S17 + 4], in0=lnS[:, SA:S17 + 4], scalar=1.0,
                in1=mask[:, SA:S17 + 4], op0=Alu.mult, op1=Alu.mult,
                accum_out=pack[:, 1:2])
            nc.scalar.activation(lnS[:, S17 + 4:S17 + 8],
                                 Sacc[:, S17 + 4:S17 + 8], Act.Ln)
            nc.vector.tensor_reduce(
                out=pack[:, 3:4], in_=dcols[:, :],
                axis=mybir.AxisListType.X, op=Alu.add)
            psF = lgap.tile([128, 1024], f32, tag="lga")
            for j in (0, 1, 3, 4, 5):
                nc.tensor.matmul(out=psF[0:1, j:j + 1],
                                 lhsT=pack[:, j:j + 1],
                                 rhs=onesc[:, 0:1], start=True, stop=True)
            nc.vector.scalar_tensor_tensor(
                out=scrln[:, S17 + 4:S17 + 8], in0=lnS[:, S17 + 4:S17 + 8],
                scalar=1.0, in1=mask[:, S17 + 4:S17 + 8], op0=Alu.mult,
                op1=Alu.mult, accum_out=pack[:, 2:3])
            nc.tensor.matmul(out=psF[0:1, 2:3], lhsT=pack[:, 2:3],
                             rhs=onesc[:, 0:1], start=True, stop=True)
            out_sb = st.tile([1, 8], f32, tag="out_sb")
            nc.vector.tensor_copy(out=out_sb[0:1, 0:6], in_=psF[0:1, 0:6])
            nc.sync.dma_start(out=out_d[:, :], in_=out_sb[0:1, 0:6])

    nc.finalize()
    return nc


def _get_nc():
    global _CACHED_NC
    if _CACHED_NC is None:
        _CACHED_NC = _build_nc()
    return _CACHED_NC


def _core_position_lists():
    """275 positions -> 8 cores: 3 cores x 35, 5 cores x 34."""
    lists = []
    start = 0
    for i in range(N_CORES):
        cnt = 35 if i < 3 else 34
        lists.append(list(range(start, start + cnt)))
        start += cnt
    assert start == P
    return lists


def _prep_in_maps(f_t_global, x_t_local, x_t_prev_local, m_t, m_t_prev, c_t,
                  c_t_prev, W_join, b_join):
    W = W_join.astype(np.float64)
    Wl, Wm, Wc = W[:C], W[C:C + DM], W[C + DM:]
    biasP = (m_t_prev.astype(np.float64) @ Wm
             + c_t_prev.astype(np.float64) @ Wc + b_join)
    biasT = (m_t.astype(np.float64) @ Wm
             + c_t.astype(np.float64) @ Wc + b_join)
    dP = np.linalg.solve(Wl.T, biasP.T)  # [C, N]
    dT = np.linalg.solve(Wl.T, biasT.T)

    sth0 = np.zeros((128, STH_COLS), dtype=np.float16)
    sth0[:, _H_G1:_H_G1 + N] = (Wl @ f_t_global.astype(np.float64).T
                                ).astype(np.float16)
    sth0[:, _H_M:_H_M + C] = (Wl @ Wl.T).astype(np.float16)
    sth0[:, _H_BPP:_H_BPP + N] = (Wl @ biasP.T).astype(np.float16)

    # [N, C, SY, SX] -> [P, C, N]
    locp = np.ascontiguousarray(
        x_t_prev_local.reshape(N, C, P).transpose(2, 1, 0))
    loct = np.ascontiguousarray(
        x_t_local.reshape(N, C, P).transpose(2, 1, 0))
    lt_ = (loct + dT[None].astype(np.float32)).astype(np.float16)
    lp_ = (locp + dP[None].astype(np.float32)).astype(np.float16)
    lp0 = locp.astype(np.float16)

    in_maps = []
    for ids in _core_position_lists():
        npos = len(ids)
        loc = np.zeros((N_SUPERS, 128, LOC_COLS), dtype=np.float16)
        for j, p in enumerate(ids):
            s, k = divmod(j, 2)
            loc[s, :, k * N:(k + 1) * N] = lt_[p]
            loc[s, :, 2 * N + k * N:2 * N + (k + 1) * N] = lp_[p]
            loc[s, :, 4 * N + k * N:4 * N + (k + 1) * N] = lp0[p]
        stf = np.zeros((128, STF_COLS), dtype=np.float32)
        # mask col (s*8 + b): block b is position 2s + (b & 1)
        for s in range(N_SUPERS):
            for b in range(8):
                if 2 * s + (b & 1) < npos:
                    stf[:, _F_MASK + 8 * s + b] = 1.0
        stf[:, _F_SHIFT] = -SHIFT
        stf[:, _F_ONES] = 1.0
        # hoisted diag-sum operands: position sums of the shipped fp16 data
        sth = np.zeros((128, STH_COLS + LOC_COLS), dtype=np.float16)
        sth[:, :STH_COLS] = sth0
        sth[:, _H_LT:_H_LT + N] = (lt_[ids].astype(np.float32).sum(axis=0)
                                   ).astype(np.float16)
        sth[:, _H_LP:_H_LP + N] = (lp_[ids].astype(np.float32).sum(axis=0)
                                   ).astype(np.float16)
        sth[:, STH_COLS:] = loc[0]
        in_maps.append({"loc": loc, "sth": sth, "stf": stf})
    return in_maps


def kernel(f_t_global, x_t_local, x_t_prev_local, m_t, m_t_prev, c_t,
           c_t_prev, W_join, b_join):
    from concourse.bass_utils import run_bass_kernel_spmd

    args = [f_t_global, x_t_local, x_t_prev_local, m_t, m_t_prev, c_t,
            c_t_prev, W_join, b_join]
    args = [np.asarray(a, dtype=np.float32) for a in args]
    in_maps = _prep_in_maps(*args)
    nc = _get_nc()
    res = run_bass_kernel_spmd(nc, in_maps, core_ids=list(range(N_CORES)))
    return combine(res)


def combine(res):
    """Host-side reduction of the 8 per-core [4, 1] partials."""
    total = 0.0
    for i, ids in enumerate(_core_position_lists()):
        v = res.results[i]["out"].reshape(-1)
        npos = len(ids)
        # v[0:3] = masked ln(S) pieces = sum(lse - SHIFT); v[3:6] = diag
        # sums (d2a, d2b, d1)
        total += (float(v[0]) + float(v[1]) + float(v[2])
                  + SHIFT * 2 * N * npos
                  - float(v[3]) - float(v[4]) - float(v[5]))
    return np.asarray(total / (P * N), dtype=np.float32)


# revision 80
# speedup vs baseline: 1.2575x; 1.1998x over previous
"""Distributed Trainium2 kernel for the contrastive InfoNCE loss problem.

Strategy: shard the P = SY*SX = 275 position axis across 8 NeuronCores
(36 position slots per core, zero-padded + mask-corrected), logits in
[n, m] layout (n = anchor index on partitions, m = positive index on the
free axis) so the LSE reduction is a cheap free-axis DVE reduce:

  logits1 = G1^T @ lp'          G1  = Wl @ f^T          (static)
  logits2 = lt'^T @ (M @ lp0) + lt'^T @ BPP
            M   = Wl @ Wl^T     BPP = Wl @ biasP^T      (static)

where lt' = loc_t + solve(Wl^T, biasT^T) and lp' = loc_p +
solve(Wl^T, biasP^T) carry the join biases folded into the shipped fp16
data (so no on-chip bias adds), and lp0 is the plain loc_p for the
K = M @ lp0 path (keeps the M-cancellation well-conditioned).

Per super (2 positions): 11 matmuls (K, 2x logits1, 8x logits2), one
PSUM->SBUF fp16 conversion of K on DVE, one [128, 8x256] Exp activation
on ScalarE (PSUM in, bf16 SBUF out, bias = -SHIFT), one segmented DVE
reduce -> per-(position, n-chunk, loss) sums, and three DVE
tensor_tensor_reduce ops that produce all diagonal sums directly from
the SBUF operands (diag(A^T B) summed == sum(A*B)).  Epilogue: masked
ln-sum + diag totals -> one ones-matmul partition sum -> [4,1] DRAM.
Host sums the per-core partials.
"""

import numpy as np

# Problem constants (from the nn_ALL_9320079032780 spec).
N = 256
C = 128
SY, SX = 11, 25
P = SY * SX  # 275
D = 128
DM = 64
DC = 64
N_CORES = 8
POS_PER_CORE = 36  # padded; 18 supers of 2 positions
N_SUPERS = POS_PER_CORE // 2

SHIFT = 20.0

# packed fp16 statics layout: [G1 | M | BPP | Lt | Lp]
_H_G1 = 0
_H_M = _H_G1 + N
_H_BPP = _H_M + C
_H_LT = _H_BPP + N
_H_LP = _H_LT + N
STH_COLS = _H_LP + N
# packed f32 statics layout: [mask | shift | ones]
_F_MASK = 0
_F_SHIFT = _F_MASK + 8 * N_SUPERS
_F_ONES = _F_SHIFT + 1
STF_COLS = _F_ONES + 1

# per-super loc layout: [lt'(A) | lt'(B) | lp'(A) | lp'(B) | lp0(A) | lp0(B)]
LOC_COLS = 6 * N

_CACHED_NC = None


def _build_nc():
    import concourse.bass as bass  # noqa: F401
    import concourse.mybir as mybir
    import concourse.tile as tile
    from concourse import bacc

    f32 = mybir.dt.float32
    fp16 = mybir.dt.float16
    bf16 = mybir.dt.bfloat16
    Alu = mybir.AluOpType
    Act = mybir.ActivationFunctionType

    nc = bacc.Bacc("TRN2", target_bir_lowering=False, debug=False,
                   num_devices=N_CORES)

    # Make the act-table pass pick the combined exp+ln set so the kernel
    # pays a single ACT_TABLE_LOAD instead of one per function family.
    from concourse.hw_specs import get_activation_tables
    _tabs = get_activation_tables(nc.m.arch)
    _Exp, _Ln = Act.Exp, Act.Ln
    for _name, _fns in _tabs.items():
        if _name != "natural_log_exp_and_others":
            _fns.discard(_Exp)
            _fns.discard(_Ln)

    loc_d = nc.declare_dram_parameter("loc", [N_SUPERS, 128, LOC_COLS], fp16,
                                      isOutput=False)
    # sth and super-0's loc tile ride one DMA: a single DGE latency +
    # completion semaphore on the critical startup chain instead of two.
    sth_d = nc.declare_dram_parameter("sth", [128, STH_COLS + LOC_COLS],
                                      fp16, isOutput=False)
    stf_d = nc.declare_dram_parameter("stf", [128, STF_COLS], f32,
                                      isOutput=False)
    out_d = nc.declare_dram_parameter("out", [1, 6], f32, isOutput=True)

    with tile.TileContext(nc) as tc:
        with (
            tc.tile_pool(name="statics", bufs=1) as st,
            tc.tile_pool(name="loc", bufs=8) as locpool,
            tc.tile_pool(name="ksb", bufs=8) as kpool,
            tc.tile_pool(name="et", bufs=8) as etpool,
            tc.tile_pool(name="etf", bufs=6) as etfpool,
            tc.tile_pool(name="scr", bufs=2) as scrpool,
            tc.tile_pool(name="lga", bufs=2, space="PSUM") as lgap,
            tc.tile_pool(name="lgb", bufs=2, space="PSUM") as lgbp,
        ):
            # ---- statics: two packed DMAs on the scalar queue ----
            sth = st.tile([128, STH_COLS + LOC_COLS], fp16, tag="sth")
            stf = st.tile([128, STF_COLS], f32, tag="stf")
            nc.sync.dma_start(out=sth[:, :], in_=sth_d[:, :])
            nc.scalar.dma_start(out=stf[:, :], in_=stf_d[:, :])
            loc0 = sth[:, STH_COLS:STH_COLS + LOC_COLS]
            G1 = sth[:, _H_G1:_H_G1 + N]
            Mst = sth[:, _H_M:_H_M + C]
            BPP = sth[:, _H_BPP:_H_BPP + N]
            Lt = sth[:, _H_LT:_H_LT + N]
            Lp = sth[:, _H_LP:_H_LP + N]
            mask = stf[:, _F_MASK:_F_MASK + 8 * N_SUPERS]
            shiftc = stf[:, _F_SHIFT:_F_SHIFT + 1]
            onesc = stf[:, _F_ONES:_F_ONES + 1]

            # persistent accumulators (bf16 lse sums: 2-byte dst enables the
            # DVE 2x perf mode on the reduce; rounding is within tolerance)
            Sacc = st.tile([128, 8 * N_SUPERS], bf16, tag="Sacc")
            dcols = st.tile([128, N_SUPERS], f32, tag="dcols")
            lnS = st.tile([128, 8 * N_SUPERS], f32, tag="lnS")
            scrln = st.tile([128, 8 * N_SUPERS], f32, tag="scrln")
            scr2 = st.tile([128, N], fp16, tag="scr2")
            pack = st.tile([128, 6], f32, tag="pack")

            # HAM warmup: dummy matmuls keep the PE busy/ramping through
            # the cold window while the first DMAs land.
            wtile = st.tile([128, 128], fp16, tag="wtile")
            nc.vector.memset(wtile[:, :], 0.0)
            wps = lgap.tile([128, 1024], f32, tag="lga")
            for _w in range(3):
                nc.tensor.matmul(
                    out=wps[:, 0:512].rearrange("p (k n) -> p k n", k=4),
                    lhsT=wtile,
                    rhs=wtile[:, :].unsqueeze(1).broadcast_to([128, 4, 128]),
                    start=True, stop=True)
            # prime the act table (exp+ln combined set) with no dependency
            # on the statics DMA so the ~1.3us load runs during the fill.
            prim = st.tile([128, 1], f32, tag="prim")
            nc.scalar.activation(prim[:, :], wtile[:, 0:1], Act.Exp,
                                 bias=0.0)

            def stage_dma(s):
                lpt = locpool.tile([128, LOC_COLS], fp16, tag="lpt")
                nc.sync.dma_start(out=lpt[:, :], in_=loc_d[s, :, :])
                return lpt

            def emit_k_mm(s, lpt):
                # K = M @ lp0 into bank0 of this super's lgB tile — l2
                # overwrites it only after the cast read it, and l2 depends
                # on the cast output anyway, so no extra serialization.
                lgb = lgbp.tile([128, 1024], f32, tag="lgb")
                nc.tensor.matmul(out=lgb[:, 0:512], lhsT=Mst,
                                 rhs=lpt[:, 4 * N:6 * N], start=True,
                                 stop=True)
                return lgb

            def emit_cast(s, lgb, on_act):
                # K PSUM -> fp16 SBUF (split DVE / ScalarE by super).
                ksb = kpool.tile([128, 512], fp16, tag="ksb")
                if on_act:
                    nc.scalar.copy(out=ksb[:, :], in_=lgb[:, 0:512])
                else:
                    nc.vector.tensor_copy(out=ksb[:, :], in_=lgb[:, 0:512])
                return ksb

            def emit_l1(s, lpt):
                # logits1: lhsT = G1 chunk, rhs = lp' (both positions).
                lga = lgap.tile([128, 1024], f32, tag="lga")
                lp = lpt[:, 2 * N:4 * N]
                for h in range(2):
                    nc.tensor.matmul(out=lga[:, h * 512:(h + 1) * 512],
                                     lhsT=G1[:, h * 128:(h + 1) * 128],
                                     rhs=lp, start=True, stop=True)
                return lga

            def emit_l2(s, lpt, lgb, ksb, k):
                # logits2 position k: lhsT = lt' n-chunk, rhs = K then BPP.
                for h in range(2):
                    ob = lgb[:, h * 512 + k * 256:h * 512 + (k + 1) * 256]
                    lh = lpt[:, k * N + h * 128:k * N + (h + 1) * 128]
                    nc.tensor.matmul(out=ob, lhsT=lh,
                                     rhs=ksb[:, k * N:(k + 1) * N],
                                     start=True, stop=False)
                    nc.tensor.matmul(out=ob, lhsT=lh, rhs=BPP,
                                     start=False, stop=True)

            def emit_exp_a(s, lga):
                et = etpool.tile([128, 2048], bf16, tag="et")
                nc.scalar.activation(et[:, 0:1024], lga[:, :], Act.Exp,
                                     bias=shiftc[:, 0:1])
                return et

            def emit_exp_b(s, lgb, et):
                nc.scalar.activation(et[:, 1024:2048], lgb[:, :], Act.Exp,
                                     bias=shiftc[:, 0:1])

            def emit_fold(s, et):
                # GpSimd folds the two 128-halves of each block, halving the
                # DVE reduce width; every 3rd super gets a second fold.
                etf = etfpool.tile([128, 1024], bf16, tag="etf")
                ev = et[:, :].rearrange("p (q t m) -> p q t m", q=8, t=2)
                nc.gpsimd.tensor_tensor(
                    out=etf[:, :].rearrange("p (q m) -> p q m", q=8),
                    in0=ev[:, :, 0, :], in1=ev[:, :, 1, :], op=Alu.add)
                return (etf, 128)

            def emit_red(s, etf_m):
                etf, m = etf_m
                with nc.allow_low_precision("bf16 lse sums within tolerance"):
                    nc.vector.tensor_reduce(
                        out=Sacc[:, 8 * s:8 * (s + 1)],
                        in_=etf[:, :].rearrange("p (q m) -> p q m", q=8),
                        axis=mybir.AxisListType.X, op=Alu.add)

            def emit_diags(s, lpt, ksb):
                # diag2 K-part via sum(lt' * K) == sum(diag(pred^T pos_nb));
                # the static-side diag sums (lt'*BPP, lp'*G1) are hoisted to
                # the host (Lt/Lp statics, epilogue).
                scr = scrpool.tile([128, 512], fp16, tag="scr")
                nc.vector.scalar_tensor_tensor(
                    out=scr[:, :], in0=lpt[:, 0:2 * N], scalar=1.0,
                    in1=ksb[:, :], op0=Alu.mult, op1=Alu.mult,
                    accum_out=dcols[:, s:s + 1])

            # ---- main loop, software-pipelined ----
            # PE order per iter: mmK(s+1) first (its lgA buffer was freed by
            # expA(s-1) early last iter), then l1(s), l2(s) — so the cast of
            # K(s+1) has a full iteration of slack before l1(s+1) needs it.
            lpt_cur = loc0
            lpt_nxt = stage_dma(1)
            lgb_cur = emit_k_mm(0, lpt_cur)
            ksb_cur = emit_cast(0, lgb_cur, on_act=False)
            etfs = []
            for s in range(N_SUPERS):
                if s + 2 < N_SUPERS:
                    lpt_fut = stage_dma(s + 2)
                lga_cur = emit_l1(s, lpt_cur)
                et_cur = emit_exp_a(s, lga_cur)
                if s + 1 < N_SUPERS:
                    lgb_nxt = emit_k_mm(s + 1, lpt_nxt)
                    ksb_nxt = emit_cast(s + 1, lgb_nxt,
                                        on_act=(s % 2 == 0))
                emit_l2(s, lpt_cur, lgb_cur, ksb_cur, 0)
                emit_l2(s, lpt_cur, lgb_cur, ksb_cur, 1)
                emit_exp_b(s, lgb_cur, et_cur)
                # reduce before the diag stt: the 1-port reduce is immune to
                # the GpSimd SBUF-port contention while the fold runs; the
                # 2-port stt is lagged one super so it lands after the
                # fold's contention window.
                if len(etfs) >= 2:
                    emit_red(s - 2, etfs[-2])
                if s >= 1:
                    emit_diags(s - 1, lpt_prv, ksb_prv)
                lpt_prv, ksb_prv = lpt_cur, ksb_cur
                if s < N_SUPERS - 1:
                    etfs.append(emit_fold(s, et_cur))
                else:
                    et_last = et_cur
                if s == N_SUPERS - 2:
                    # partial epilogue for supers 0..13 (finalized by
                    # red(13), emitted at s=15) overlaps the last supers.
                    SA = 8 * 14
                    nc.scalar.activation(lnS[:, 0:SA], Sacc[:, 0:SA],
                                         Act.Ln)
                    nc.vector.scalar_tensor_tensor(
                        out=scrln[:, 0:SA], in0=lnS[:, 0:SA], scalar=1.0,
                        in1=mask[:, 0:SA], op0=Alu.mult, op1=Alu.mult,
                        accum_out=pack[:, 0:1])
                    nc.vector.scalar_tensor_tensor(
                        out=scr2[:, :], in0=Lt, scalar=1.0, in1=BPP,
                        op0=Alu.mult, op1=Alu.mult, accum_out=pack[:, 4:5])
                    nc.vector.scalar_tensor_tensor(
                        out=scr2[:, :], in0=Lp, scalar=1.0, in1=G1,
                        op0=Alu.mult, op1=Alu.mult, accum_out=pack[:, 5:6])
                if s + 1 < N_SUPERS:
                    lpt_cur, lgb_cur, ksb_cur = lpt_nxt, lgb_nxt, ksb_nxt
                    if s + 2 < N_SUPERS:
                        lpt_nxt = lpt_fut
            emit_diags(N_SUPERS - 1, lpt_prv, ksb_prv)
            emit_red(N_SUPERS - 2, etfs[-1])
            # last super: direct (unfolded) reduce in two halves to skip the
            # Pool hop in the drain; the A half overlaps expB(17).
            S17 = 8 * (N_SUPERS - 1)
            with nc.allow_low_precision("bf16 lse sums within tolerance"):
                nc.vector.tensor_reduce(
                    out=Sacc[:, S17:S17 + 4],
                    in_=et_last[:, 0:1024].rearrange("p (q m) -> p q m", q=4),
                    axis=mybir.AxisListType.X, op=Alu.add)
                # lnB1 covers supers 14-16 plus super 17's l1 blocks — only
                # the last 4 l2 columns serialize behind the final reduce.
                SA = 8 * 14
                nc.scalar.activation(lnS[:, SA:S17 + 4], Sacc[:, SA:S17 + 4],
                                     Act.Ln)
                nc.vector.tensor_reduce(
                    out=Sacc[:, S17 + 4:S17 + 8],
                    in_=et_last[:, 1024:2048].rearrange("p (q m) -> p q m",
                                                        q=4),
                    axis=mybir.AxisListType.X, op=Alu.add)
            nc.vector.scalar_tensor_tensor(
                out=scrln[:, SA:S17 + 4], in0=lnS[:, SA:S17 + 4], scalar=1.0,
                in1=mask[:, SA:S17 + 4], op0=Alu.mult, op1=Alu.mult,
                accum_out=pack[:, 1:2])
            nc.scalar.activation(lnS[:, S17 + 4:S17 + 8],
                                 Sacc[:, S17 + 4:S17 + 8], Act.Ln)
            nc.vector.tensor_reduce(
                out=pack[:, 3:4], in_=dcols[:, :],
                axis=mybir.AxisListType.X, op=Alu.add)
            psF = lgap.tile([128, 1024], f32, tag="lga")
            for j in (0, 1, 3, 4, 5):
                nc.tensor.matmul(out=psF[0:1, j:j + 1],
                                 lhsT=pack[:, j:j + 1],
                                 rhs=onesc[:, 0:1], start=True, stop=True)
            nc.vector.scalar_tensor_tensor(
                out=scrln[:, S17 + 4:S17 + 8], in0=lnS[:, S17 + 4:S17 + 8],
                scalar=1.0, in1=mask[:, S17 + 4:S17 + 8], op0=Alu.mult,
                op1=Alu.mult, accum_out=pack[:, 2:3])
            nc.tensor.matmul(out=psF[0:1, 2:3], lhsT=pack[:, 2:3],
                             rhs=onesc[:, 0:1], start=True, stop=True)
            out_sb = st.tile([1, 8], f32, tag="out_sb")
            nc.vector.tensor_copy(out=out_sb[0:1, 0:6], in_=psF[0:1, 0:6])
            nc.sync.dma_start(out=out_d[:, :], in_=out_sb[0:1, 0:6])

    nc.finalize()
    return nc


def _get_nc():
    global _CACHED_NC
    if _CACHED_NC is None:
        _CACHED_NC = _build_nc()
    return _CACHED_NC


def _core_position_lists():
    """275 positions -> 8 cores: 3 cores x 35, 5 cores x 34."""
    lists = []
    start = 0
    for i in range(N_CORES):
        cnt = 35 if i < 3 else 34
        lists.append(list(range(start, start + cnt)))
        start += cnt
    assert start == P
    return lists


def _prep_in_maps(f_t_global, x_t_local, x_t_prev_local, m_t, m_t_prev, c_t,
                  c_t_prev, W_join, b_join):
    W = W_join.astype(np.float64)
    Wl, Wm, Wc = W[:C], W[C:C + DM], W[C + DM:]
    biasP = (m_t_prev.astype(np.float64) @ Wm
             + c_t_prev.astype(np.float64) @ Wc + b_join)
    biasT = (m_t.astype(np.float64) @ Wm
             + c_t.astype(np.float64) @ Wc + b_join)
    dP = np.linalg.solve(Wl.T, biasP.T)  # [C, N]
    dT = np.linalg.solve(Wl.T, biasT.T)

    sth0 = np.zeros((128, STH_COLS), dtype=np.float16)
    sth0[:, _H_G1:_H_G1 + N] = (Wl @ f_t_global.astype(np.float64).T
                                ).astype(np.float16)
    sth0[:, _H_M:_H_M + C] = (Wl @ Wl.T).astype(np.float16)
    sth0[:, _H_BPP:_H_BPP + N] = (Wl @ biasP.T).astype(np.float16)

    # [N, C, SY, SX] -> [P, C, N]
    locp = np.ascontiguousarray(
        x_t_prev_local.reshape(N, C, P).transpose(2, 1, 0))
    loct = np.ascontiguousarray(
        x_t_local.reshape(N, C, P).transpose(2, 1, 0))
    lt_ = (loct + dT[None].astype(np.float32)).astype(np.float16)
    lp_ = (locp + dP[None].astype(np.float32)).astype(np.float16)
    lp0 = locp.astype(np.float16)

    in_maps = []
    for ids in _core_position_lists():
        npos = len(ids)
        loc = np.zeros((N_SUPERS, 128, LOC_COLS), dtype=np.float16)
        for j, p in enumerate(ids):
            s, k = divmod(j, 2)
            loc[s, :, k * N:(k + 1) * N] = lt_[p]
            loc[s, :, 2 * N + k * N:2 * N + (k + 1) * N] = lp_[p]
            loc[s, :, 4 * N + k * N:4 * N + (k + 1) * N] = lp0[p]
        stf = np.zeros((128, STF_COLS), dtype=np.float32)
        # mask col (s*8 + b): block b is position 2s + (b & 1)
        for s in range(N_SUPERS):
            for b in range(8):
                if 2 * s + (b & 1) < npos:
                    stf[:, _F_MASK + 8 * s + b] = 1.0
        stf[:, _F_SHIFT] = -SHIFT
        stf[:, _F_ONES] = 1.0
        # hoisted diag-sum operands: position sums of the shipped fp16 data
        sth = np.zeros((128, STH_COLS + LOC_COLS), dtype=np.float16)
        sth[:, :STH_COLS] = sth0
        sth[:, _H_LT:_H_LT + N] = (lt_[ids].astype(np.float32).sum(axis=0)
                                   ).astype(np.float16)
        sth[:, _H_LP:_H_LP + N] = (lp_[ids].astype(np.float32).sum(axis=0)
                                   ).astype(np.float16)
        sth[:, STH_COLS:] = loc[0]
        in_maps.append({"loc": loc, "sth": sth, "stf": stf})
    return in_maps


def kernel(f_t_global, x_t_local, x_t_prev_local, m_t, m_t_prev, c_t,
           c_t_prev, W_join, b_join):
    from concourse.bass_utils import run_bass_kernel_spmd

    args = [f_t_global, x_t_local, x_t_prev_local, m_t, m_t_prev, c_t,
            c_t_prev, W_join, b_join]
    args = [np.asarray(a, dtype=np.float32) for a in args]
    in_maps = _prep_in_maps(*args)
    nc = _get_nc()
    res = run_bass_kernel_spmd(nc, in_maps, core_ids=list(range(N_CORES)))
    return combine(res)


def combine(res):
    """Host-side reduction of the 8 per-core [4, 1] partials."""
    total = 0.0
    for i, ids in enumerate(_core_position_lists()):
        v = res.results[i]["out"].reshape(-1)
        npos = len(ids)
        # v[0:3] = masked ln(S) pieces = sum(lse - SHIFT); v[3:6] = diag
        # sums (d2a, d2b, d1)
        total += (float(v[0]) + float(v[1]) + float(v[2])
                  + SHIFT * 2 * N * npos
                  - float(v[3]) - float(v[4]) - float(v[5]))
    return np.asarray(total / (P * N), dtype=np.float32)
